# revision 34
# baseline (speedup 1.0000x reference)
"""TRN2 Bass kernel for 2-layer GAT + grouped softmax (nn_Actor_1881195675935).

8-core SPMD, nodes sharded contiguously (12500/core, padded to 12544 = 98
tiles of 128); edges live with the owner of their dst node in an ELLPACK
layout (partition = dst node, free axis = edge slots).

v4 design (vs v3): the v3 trace showed the gather descriptor stream draining
at ~2.3 ns/desc — the phases are bound by descriptor count, not DVE.  v4 cuts
descriptors 19% and removes the two-window machinery entirely:
  * per-head feature-basis rotation: rows store h' = Q_h h with Q_h chosen so
    a_src = s_h * h'[0].  Node rows shrink to 32 bf16 (64B), so a 256B gather
    row holds FOUR nodes and quad indices (pos>>2 < 25088) fit int16 in a
    SINGLE window — ELLPACK needs no A/B split (totWT 4032 -> 3270, one
    gather call per tile).  Aggregated num' is un-rotated per tile with a
    tiny PE matmul (blockdiag(Q_h)) before bias+relu.
  * logits come straight off the gathered bytes on the ACT engine:
    e = exp(a's * s_h + a_d) via activation(EXP, scale=s_h, bias=ad_ap) — no
    DVE z materialisation; 4-way sub-row masks (multiplicative, bf16) pick
    the real half.
  * both layers share one resident gidx/mask set; AllGathers halve (6.4MB).
All v3 scheduling fixes kept: no 2-port DVE ops in gather phases, ACT-engine
casts (one act-table set), batched HWDGE transfers.
"""

import sys

sys.path.insert(0, "/opt/trn_rl_repo")

import numpy as np
import ml_dtypes  # noqa: F401

N = 100000
NPC_REAL = 12500
NPC = 12544               # = 98 * 128
NT = 98
NC = 8
F_IN = 128
H1, C1 = 2, 16
OUT = 16
WSEG = 256
ROW = 32                  # bf16 elements per node row (64B); quad row = 256B
NQROWS = NC * NPC // 4    # 25088 quad rows (single int16 window)
CHUNK = 6                 # max slot columns per dma_gather call (768 descs;
                          # two calls fit one SWDGE queue ring, so Q7 never
                          # blocks inside a call and all 4 queues drain in
                          # parallel)
PADQ = (2 * NPC + NPC_REAL) // 4   # all-zero pad quad (core 2 spares)


def _orth_rot(a):
    """Orthogonal Q (16x16) with Q[0,:] ~ a/||a||; returns (Q, s) such that
    a @ h == s * (Q @ h)[0] for all h."""
    a = np.asarray(a, np.float64)
    M = np.eye(len(a))
    M[:, 0] = a
    Qc, _ = np.linalg.qr(M)
    s = float(Qc[:, 0] @ a)
    return Qc.T.astype(np.float64), s


def _preprocess(x, edge_index, index, W1, att_src1, att_dst1, b1,
                W2, att_src2, att_dst2, b2):
    f32 = np.float32
    src = np.asarray(edge_index[0], dtype=np.int64)
    dst = np.asarray(edge_index[1], dtype=np.int64)
    loops = np.arange(N, dtype=np.int64)
    src = np.concatenate([src, loops]).astype(np.int64)
    dst = np.concatenate([dst, loops]).astype(np.int64)

    owner_dst = dst // NPC_REAL
    ldst = dst - owner_dst * NPC_REAL

    # per-core relabel: degree-sorted descending so each tile's 128 lanes have
    # near-equal degree (minimises ELLPACK padding via max-over-lanes)
    counts_deg = np.bincount(owner_dst * NPC + ldst, minlength=NC * NPC)
    counts_deg = counts_deg.reshape(NC, NPC)
    orders = np.zeros((NC, NPC), dtype=np.int64)
    for c in range(NC):
        orders[c] = np.argsort(-counts_deg[c], kind="stable")
    inv_orders = np.argsort(orders, axis=1)
    pos = np.zeros(N, dtype=np.int64)
    ar = np.arange(NPC_REAL)
    for c in range(NC):
        pos[c * NPC_REAL + ar] = c * NPC + inv_orders[c][ar]

    spos = pos[src]
    nid = owner_dst * NPC + inv_orders[owner_dst, ldst]

    deg = np.bincount(nid, minlength=NC * NPC)
    WT = deg.reshape(NC, NT, 128).max(axis=(0, 2)).astype(np.int64)  # [NT]
    totWT = int(WT.sum())
    wt_off = np.concatenate([[0], np.cumsum(WT)]).astype(np.int64)

    calls = []  # (tile, col0, ncols)
    for t in range(NT):
        c0 = 0
        while c0 < int(WT[t]):
            cn = min(CHUNK, int(WT[t]) - c0)
            calls.append((t, c0, cn))
            c0 += cn
    idxw_off = [0]
    for (_t, _c0, cn) in calls:
        idxw_off.append(idxw_off[-1] + 8 * cn)
    IDXW = idxw_off[-1]

    # sorted edge runs per node
    eorder = np.lexsort((spos, nid))
    s_spos = spos[eorder]
    run_starts = np.zeros(NC * NPC + 1, dtype=np.int64)
    np.cumsum(deg, out=run_starts[1:])

    bf16 = ml_dtypes.bfloat16
    gidx = np.zeros((NC, 128, IDXW), dtype=np.int16)
    # per-slot sub-row selector (-1 = unused slot); device materialises the
    # 4-way one-hot mask via is_equal against the qv constant
    sel = np.full((NC, 128, totWT), -1.0, dtype=np.float32)

    for c in range(NC):
        nodes_all = c * NPC + np.arange(NPC)
        r0 = run_starts[nodes_all]
        d_all = deg[nodes_all]
        for t in range(NT):
            wt = int(WT[t])
            flat = np.full((wt, 128), PADQ, dtype=np.int64)
            sub = np.zeros((wt, 128), dtype=np.int64)
            used = np.zeros((wt, 128), dtype=bool)
            for lane in range(128):
                n = t * 128 + lane
                d = int(d_all[n])
                if d == 0:
                    continue
                e0 = r0[n]
                sp = s_spos[e0:e0 + d]
                flat[:d, lane] = sp >> 2
                sub[:d, lane] = sp & 3
                used[:d, lane] = True
            sv = np.full((wt, 128), -1.0, dtype=np.float32)
            sv[used] = sub[used]
            dead = ~used.any(axis=0)
            sv[0, dead] = 0.0  # pad quad, sub 0: zero node -> den>0, num+=0
            sel[c, :, wt_off[t]:wt_off[t] + wt] = sv.T
            assert flat.max() < NQROWS and flat.min() >= 0
            for ci, (tt, col0, cn) in enumerate(calls):
                if tt != t:
                    continue
                blk = flat[col0:col0 + cn]
                w16 = blk.reshape(-1, 16).T.astype(np.int16)
                gidx[c, :, idxw_off[ci]:idxw_off[ci + 1]] = np.tile(w16, (8, 1))

    W1 = np.asarray(W1, np.float64); W2 = np.asarray(W2, np.float64)
    as1 = np.asarray(att_src1, np.float64); ad1 = np.asarray(att_dst1, np.float64)
    as2 = np.asarray(att_src2, np.float64); ad2 = np.asarray(att_dst2, np.float64)
    Q1 = []; s1 = []
    for h in range(H1):
        Qh, sh = _orth_rot(as1[h])
        Q1.append(Qh); s1.append(sh)
    W1q = np.concatenate(
        [W1[:, h * C1:(h + 1) * C1] @ Q1[h].T for h in range(H1)], axis=1)
    vd1 = np.stack([W1[:, h * C1:(h + 1) * C1] @ ad1[h] for h in range(H1)], 1)
    wcat1 = np.concatenate([W1q, vd1], axis=1).astype(bf16)      # [128, 34]
    qb1 = np.zeros((2 * C1, 2 * C1))
    for h in range(H1):
        qb1[h * C1:(h + 1) * C1, h * C1:(h + 1) * C1] = Q1[h]
    qb1 = qb1.astype(bf16)                                       # [32, 32]

    Q2, s2 = _orth_rot(as2[0])
    W2q = W2 @ Q2.T
    vd2 = (W2 @ ad2[0])[:, None]
    wcat2 = np.concatenate([W2q, vd2], axis=1).astype(bf16)      # [32, 17]
    qm2 = Q2.astype(bf16)                                        # [16, 16]

    x = np.asarray(x, f32)
    xT = np.zeros((NC, F_IN, NPC), dtype=bf16)
    glb = np.zeros((NC, NPC), dtype=np.int64)
    real = np.zeros((NC, NPC), dtype=bool)
    for c in range(NC):
        ol = orders[c]
        is_real = ol < NPC_REAL
        g = np.where(is_real, c * NPC_REAL + np.minimum(ol, NPC_REAL - 1), 0)
        xT[c] = np.where(is_real[:, None], x[g], 0.0).T.astype(bf16)
        glb[c] = g
        real[c] = is_real

    index = np.asarray(index, np.int64)
    seg = np.zeros((NC, NPC), dtype=np.int64)
    g0 = np.zeros(NC, dtype=np.int64)
    for c in range(NC):
        seg[c] = np.where(real[c], index[glb[c]], 0)
        s = seg[c][real[c]]
        g0[c] = s.min()
        assert s.max() - s.min() < WSEG, "segment window exceeds WSEG"
    f8 = ml_dtypes.float8_e4m3
    ohf = np.zeros((NC, NT * 128, WSEG), dtype=f8)
    oht = np.zeros((NC, NT * 128, WSEG), dtype=f8)
    for c in range(NC):
        for t in range(NT):
            sl = seg[c, t * 128:(t + 1) * 128] - g0[c]
            m = real[c, t * 128:(t + 1) * 128]
            oh = np.zeros((128, WSEG), dtype=np.float32)
            oh[np.arange(128)[m], sl[m]] = 1.0
            ohf[c, t * 128:(t + 1) * 128] = oh.astype(f8)
            ohtk = np.concatenate([oh[:, :128].T, oh[:, 128:].T], axis=1)
            oht[c, t * 128:(t + 1) * 128] = ohtk.astype(f8)

    sidx = np.zeros((NC, 128, 2), dtype=np.int32)
    for c in range(NC):
        for k in range(2):
            sidx[c, :, k] = g0[c] + k * 128 + np.arange(128)

    WTMAX = int(WT.max())
    qv = np.tile(np.arange(4, dtype=np.float32), WTMAX)
    qv = np.tile(qv[None, :], (128, 1)).astype(bf16)
    b1t = np.tile(np.asarray(b1, f32)[None, :], (128, 1)).astype(f32)
    b2t = np.tile(np.asarray(b2, f32)[None, :], (128, 1)).astype(f32)

    per_core = [{
        "xT": np.ascontiguousarray(xT[c]),
        "wcat1": wcat1, "wcat2": wcat2, "qb1": qb1, "qm2": qm2,
        "b1t": b1t, "b2t": b2t,
        "gidx": np.ascontiguousarray(gidx[c]),
        "sel": np.ascontiguousarray(sel[c].astype(bf16)),
        "qv": qv,
        "ohf": np.ascontiguousarray(ohf[c]),
        "oht": np.ascontiguousarray(oht[c]),
        "sidx": np.ascontiguousarray(sidx[c]),
    } for c in range(NC)]
    shared = {"WT": WT, "calls": calls, "idxw_off": idxw_off,
              "IDXW": IDXW, "wt_off": wt_off, "totWT": totWT,
              "s1": s1, "s2": s2, "WTMAX": int(WT.max())}
    asm = {"glb": glb, "real": real}
    return shared, per_core, asm


def _build(shared):
    import concourse.bass as bass
    import concourse.bacc as bacc
    import concourse.tile as tile
    from concourse import mybir, library_config
    from concourse.masks import make_identity

    calls = shared["calls"]
    idxw_off = shared["idxw_off"]; IDXW = shared["IDXW"]
    wt_off = shared["wt_off"]; totWT = shared["totWT"]; WT = shared["WT"]
    s1 = shared["s1"]; s2 = shared["s2"]
    f32 = mybir.dt.float32
    bf16 = mybir.dt.bfloat16
    f8 = mybir.dt.float8e4
    i16 = mybir.dt.int16
    AL = mybir.AluOpType
    EXP = mybir.ActivationFunctionType.Exp
    CPY = mybir.ActivationFunctionType.Copy
    RELU = mybir.ActivationFunctionType.Relu
    IOA = bass.IndirectOffsetOnAxis
    XAX = mybir.AxisListType.X

    nc = bacc.Bacc("TRN2", target_bir_lowering=False, debug=False,
                   num_devices=NC, num_swdge_queues=4)

    xT_ext = nc.dram_tensor("xT", [F_IN, NPC], bf16, kind="ExternalInput")
    wcat1_ext = nc.dram_tensor("wcat1", [F_IN, 34], bf16, kind="ExternalInput")
    wcat2_ext = nc.dram_tensor("wcat2", [32, 17], bf16, kind="ExternalInput")
    qb1_ext = nc.dram_tensor("qb1", [32, 32], bf16, kind="ExternalInput")
    qm2_ext = nc.dram_tensor("qm2", [16, 16], bf16, kind="ExternalInput")
    b1_ext = nc.dram_tensor("b1t", [128, 32], f32, kind="ExternalInput")
    b2_ext = nc.dram_tensor("b2t", [128, 16], f32, kind="ExternalInput")
    gidx_ext = nc.dram_tensor("gidx", [128, IDXW], i16, kind="ExternalInput")
    sel_ext = nc.dram_tensor("sel", [128, totWT], bf16, kind="ExternalInput")
    qv_ext = nc.dram_tensor("qv", [128, 4 * shared["WTMAX"]], bf16,
                            kind="ExternalInput")
    ohf_ext = nc.dram_tensor("ohf", [NT * 128, WSEG], f8, kind="ExternalInput")
    oht_ext = nc.dram_tensor("oht", [NT * 128, WSEG], f8, kind="ExternalInput")
    sidx_ext = nc.dram_tensor("sidx", [128, 2], mybir.dt.int32, kind="ExternalInput")
    out_ext = nc.dram_tensor("out", [NPC, OUT], f32, kind="ExternalOutput")

    with tile.TileContext(nc) as tc:
        with (
            tc.tile_pool(name="dram", bufs=1, space="DRAM") as dr,
            tc.tile_pool(name="const", bufs=1) as cpool,
            tc.tile_pool(name="res", bufs=1) as rp,
            tc.tile_pool(name="gat", bufs=7) as gp,
            tc.tile_pool(name="gix", bufs=3) as gxp,
            tc.tile_pool(name="logit", bufs=3) as lp,
            tc.tile_pool(name="big", bufs=2) as bp,
            tc.tile_pool(name="tiny", bufs=4) as sb,
            tc.tile_pool(name="chunk", bufs=2) as ch,
            tc.tile_pool(name="psum", bufs=2, space="PSUM") as pp,
            tc.tile_pool(name="psum_seg", bufs=1, space="PSUM") as pseg,
        ):
            tab1_loc = dr.tile([NPC, ROW], bf16, name="tab1_loc")
            tab2_loc = dr.tile([NPC, ROW], bf16, name="tab2_loc")
            tab1_full = dr.tile([NC * NPC, ROW], bf16, name="tab1_full",
                                addr_space="Shared")
            tab2_full = dr.tile([NC * NPC, ROW], bf16, name="tab2_full",
                                addr_space="Shared")
            s_loc = dr.tile([1280, OUT], f32, name="s_loc")
            s_red = dr.tile([1280, OUT], f32, name="s_red", addr_space="Shared")

            tab1v = tab1_full[:].rearrange("(r four) c -> r (four c)", four=4)
            tab2v = tab2_full[:].rearrange("(r four) c -> r (four c)", four=4)

            nc.gpsimd.load_library(library_config.mlp)

            ident = cpool.tile([128, 128], f32, name="ident")
            make_identity(nc, ident[:])
            identb = cpool.tile([128, 128], bf16, name="identb")
            nc.scalar.activation(out=identb[:], in_=ident[:], func=CPY)
            wc1 = cpool.tile([F_IN, 34], bf16, name="wc1")
            nc.sync.dma_start(out=wc1[:], in_=wcat1_ext[:, :])
            wc2 = cpool.tile([32, 17], bf16, name="wc2")
            nc.sync.dma_start(out=wc2[:], in_=wcat2_ext[:, :])
            qb1 = cpool.tile([32, 32], bf16, name="qb1")
            nc.sync.dma_start(out=qb1[:], in_=qb1_ext[:, :])
            qm2 = cpool.tile([16, 16], bf16, name="qm2")
            nc.sync.dma_start(out=qm2[:], in_=qm2_ext[:, :])
            b1s = cpool.tile([128, 32], f32, name="b1s")
            nc.sync.dma_start(out=b1s[:], in_=b1_ext[:, :])
            b2s = cpool.tile([128, 16], f32, name="b2s")
            nc.sync.dma_start(out=b2s[:], in_=b2_ext[:, :])

            # residents
            sel_all = rp.tile([128, totWT], bf16, name="sel_all")
            nc.sync.dma_start(out=sel_all[:], in_=sel_ext[:, :])
            qvc = cpool.tile([128, 4 * shared["WTMAX"]], bf16, name="qvc")
            nc.sync.dma_start(out=qvc[:], in_=qv_ext[:, :])
            stage_all = rp.tile([128, NT * ROW], bf16, name="stage_all")
            nc.vector.memset(stage_all[:], 0.0)
            ad1_all = rp.tile([128, NT * 2], f32, name="ad1_all")
            ad1b_all = rp.tile([128, NT * 2], f32, name="ad1b_all")
            ad2_all = rp.tile([128, NT], f32, name="ad2_all")
            ad2b_all = rp.tile([128, NT], f32, name="ad2b_all")
            x2_all = rp.tile([128, NT * 32], bf16, name="x2_all")
            e_all = rp.tile([128, NT * OUT], f32, name="e_all")
            ebf_all = rp.tile([128, NT * OUT], bf16, name="ebf_all")
            fo_all = rp.tile([128, NT * OUT], f32, name="fo_all")

            calls_by_tile = {}
            for ci, (t, col0, cn) in enumerate(calls):
                calls_by_tile.setdefault(t, []).append((ci, col0, cn))
            qctr = [0]

            # gidx streamed by 7-tile groups (double-prefetched)
            GRP = 7
            NGRP = (NT + GRP - 1) // GRP
            grp_lo, grp_hi = [], []
            for g in range(NGRP):
                t0, t1 = g * GRP, min((g + 1) * GRP, NT) - 1
                grp_lo.append(idxw_off[calls_by_tile[t0][0][0]])
                grp_hi.append(idxw_off[calls_by_tile[t1][-1][0] + 1])
            GW = max(h - l for l, h in zip(grp_lo, grp_hi))
            gtiles = {}

            def load_group(g, lname):
                if g >= NGRP or g in gtiles:
                    return
                gt = gxp.tile([128, GW], i16, name=f"gx{lname}_{g}", tag="gix")
                nc.sync.dma_start(out=gt[:, 0:grp_hi[g] - grp_lo[g]],
                                  in_=gidx_ext[:, grp_lo[g]:grp_hi[g]])
                gtiles[g] = gt

            def gather_tile(t, tabv, lname):
                wt = int(WT[t])
                g = t // GRP
                if t % GRP == 0:
                    load_group(g + 1, lname)
                gt = gtiles[g]
                lo = grp_lo[g]
                gq = gp.tile([128, wt, 4 * ROW], bf16, name=f"g{lname}_{t}",
                             tag="gq")
                for (ci, col0, cn) in calls_by_tile[t]:
                    nidx = 128 * cn
                    nc.gpsimd.dma_gather(
                        gq[:, col0:col0 + cn, :],
                        tabv[0:NQROWS, :],
                        gt[:, idxw_off[ci] - lo:idxw_off[ci + 1] - lo],
                        nidx, nidx, 4 * ROW, queue_num=qctr[0] % 4,
                        single_packet=False)
                    qctr[0] += 1
                return gq

            # ---- phase 0: layer-1 node rows -------------------------------
            XC = 7  # tiles per x chunk
            for t in range(NT):
                if t % XC == 0:
                    xt_c = ch.tile([128, XC * 128], bf16, name=f"xt{t}",
                                   tag="xtc")
                    nc.sync.dma_start(
                        out=xt_c[:],
                        in_=xT_ext[:, t * 128:(t + XC) * 128])
                hp = pp.tile([128, 34], f32, name=f"hp{t}", tag="hp")
                nc.tensor.matmul(out=hp[:],
                                 lhsT=xt_c[:, (t % XC) * 128:(t % XC + 1) * 128],
                                 rhs=wc1[:], start=True, stop=True)
                nc.scalar.activation(out=ad1_all[:, 2 * t:2 * t + 2],
                                     in_=hp[:, 32:34], func=CPY)
                nc.scalar.activation(out=ad1b_all[:, 2 * t:2 * t + 2],
                                     in_=hp[:, 32:34], func=CPY, scale=0.2)
                nc.scalar.activation(out=stage_all[:, ROW * t:ROW * t + 32],
                                     in_=hp[:, 0:32], func=CPY)

            nc.sync.dma_start(
                out=tab1_loc[:].rearrange("(t p) c -> p t c", p=128),
                in_=stage_all[:].rearrange("p (t c) -> p t c", c=ROW))
            nc.gpsimd.collective_compute(
                "AllGather", AL.bypass, replica_groups=[list(range(NC))],
                ins=[tab1_loc.opt()], outs=[tab1_full.opt()])

            # ---- phase 1: layer-1 aggregation (+ fused layer-2 rows) ------
            gtiles.clear()
            load_group(0, "1")
            load_group(1, "1")
            gqs = {0: gather_tile(0, tab1v, "1")}
            for t in range(NT):
                wt = int(WT[t])
                wo4 = 4 * int(wt_off[t])
                gq = gqs.pop(t)
                if t + 1 < NT:
                    gqs[t + 1] = gather_tile(t + 1, tab1v, "1")
                gq4 = gq[:].rearrange("p w (q c) -> p w q c", c=ROW)
                msk = lp.tile([128, wt, 4], bf16, name=f"mk1_{t}", tag="mk")
                nc.vector.tensor_tensor(
                    out=msk[:],
                    in0=qvc[:, 0:4 * wt].rearrange("p (w q) -> p w q", q=4),
                    in1=sel_all[:, int(wt_off[t]):int(wt_off[t]) + wt][
                        :, :, None].to_broadcast([128, wt, 4]),
                    op=AL.is_equal)
                e1 = lp.tile([128, 2 * wt, 4], bf16, name=f"e1a_{t}", tag="e1")
                e2 = lp.tile([128, 2 * wt, 4], bf16, name=f"e2a_{t}", tag="e2")
                for h in range(2):
                    adc = ad1_all[:, 2 * t + h:2 * t + h + 1]
                    adcb = ad1b_all[:, 2 * t + h:2 * t + h + 1]
                    asv = gq4[:, :, :, 16 * h:16 * h + 1]
                    nc.scalar.activation(
                        out=e1[:, h * wt:(h + 1) * wt, :, None], in_=asv,
                        func=EXP, scale=float(s1[h]), bias=adc)
                    nc.scalar.activation(
                        out=e2[:, h * wt:(h + 1) * wt, :, None], in_=asv,
                        func=EXP, scale=float(0.2 * s1[h]), bias=adcb)
                # exp(leaky_relu(z)) == max(exp(z), exp(0.2*z)); then mask
                nc.vector.tensor_tensor(out=e1[:], in0=e1[:], in1=e2[:],
                                        op=AL.max)
                e1h = e1[:].rearrange("p (h w) q -> p h w q", h=2)
                nc.vector.tensor_tensor(
                    out=e1h, in0=e1h,
                    in1=msk[:, None, :, :].to_broadcast([128, 2, wt, 4]),
                    op=AL.mult)
                den = sb.tile([128, 2], f32, name=f"den1_{t}", tag="den")
                nc.vector.reduce_sum(
                    out=den[:],
                    in_=e1[:].rearrange("p (h w) q -> p h (w q)", h=2),
                    axis=XAX)
                macc = bp.tile([128, wt, 32], bf16, name=f"mA1_{t}", tag="mA")
                mtmp = bp.tile([128, wt, 32], bf16, name=f"mB1_{t}", tag="mB")
                e1p = e1[:].rearrange("p (h w) q -> p w h q", h=2)
                for q in range(4):
                    dst4 = (macc if q == 0 else mtmp)[:].rearrange(
                        "p w (h f) -> p w h f", h=2)
                    nc.vector.tensor_tensor(
                        out=dst4,
                        in0=gq[:, :, q * ROW:(q + 1) * ROW].rearrange(
                            "p w (h f) -> p w h f", h=2),
                        in1=e1p[:, :, :, q:q + 1].to_broadcast([128, wt, 2, 16]),
                        op=AL.mult)
                    if q > 0:
                        nc.vector.tensor_tensor(out=macc[:], in0=macc[:],
                                                in1=mtmp[:], op=AL.add)
                num = sb.tile([128, 32], f32, name=f"num1_{t}", tag="num")
                nc.vector.reduce_sum(out=num[:],
                                     in_=macc[:].rearrange("p w f -> p f w"),
                                     axis=XAX)
                rcp = sb.tile([128, 2], f32, name=f"rcp1_{t}", tag="rcp")
                nc.vector.reciprocal(out=rcp[:], in_=den[:])
                x2t = sb.tile([128, 32], f32, name=f"x2_{t}", tag="x2t")
                for h in range(2):
                    nc.vector.tensor_tensor(
                        out=x2t[:, 16 * h:16 * h + 16],
                        in0=num[:, 16 * h:16 * h + 16],
                        in1=rcp[:, h:h + 1].to_broadcast([128, 16]),
                        op=AL.mult)
                # un-rotate: x2 = relu(Q^T num/den + b1)
                trp = pp.tile([32, 128], f32, name=f"trp{t}", tag="hp")
                nc.tensor.transpose(out=trp[:], in_=x2t[:], identity=ident[:])
                trb = sb.tile([32, 128], bf16, name=f"trb{t}", tag="trb")
                nc.scalar.activation(out=trb[:], in_=trp[:], func=CPY)
                xr = pp.tile([128, 32], f32, name=f"xr{t}", tag="dp")
                nc.tensor.matmul(out=xr[:], lhsT=trb[:], rhs=qb1[:],
                                 start=True, stop=True)
                x2i = sb.tile([128, 32], f32, name=f"x2i_{t}", tag="x2i")
                nc.vector.tensor_tensor(out=x2i[:], in0=xr[:], in1=b1s[:],
                                        op=AL.add)
                nc.scalar.activation(out=x2_all[:, 32 * t:32 * (t + 1)],
                                     in_=x2i[:], func=RELU)
                # fused layer-2 node-row computation for this tile
                x2tp = pp.tile([32, 128], bf16, name=f"x2tp{t}", tag="hp")
                nc.tensor.transpose(out=x2tp[:],
                                    in_=x2_all[:, 32 * t:32 * (t + 1)],
                                    identity=identb[:])
                x2ts = sb.tile([32, 128], bf16, name=f"x2ts{t}", tag="trb")
                nc.scalar.activation(out=x2ts[:], in_=x2tp[:], func=CPY)
                h2p = pp.tile([128, 17], f32, name=f"h2p{t}", tag="dp")
                nc.tensor.matmul(out=h2p[:], lhsT=x2ts[:], rhs=wc2[:],
                                 start=True, stop=True)
                nc.scalar.activation(out=ad2_all[:, t:t + 1],
                                     in_=h2p[:, 16:17], func=CPY)
                nc.scalar.activation(out=ad2b_all[:, t:t + 1],
                                     in_=h2p[:, 16:17], func=CPY, scale=0.2)
                nc.scalar.activation(out=stage_all[:, ROW * t:ROW * t + 16],
                                     in_=h2p[:, 0:16], func=CPY)


            nc.sync.dma_start(
                out=tab2_loc[:].rearrange("(t p) c -> p t c", p=128),
                in_=stage_all[:].rearrange("p (t c) -> p t c", c=ROW))
            nc.gpsimd.collective_compute(
                "AllGather", AL.bypass, replica_groups=[list(range(NC))],
                ins=[tab2_loc.opt()], outs=[tab2_full.opt()])

            # ---- phase 4 setup (hoisted; overlaps phase 3) -----------------
            zt = sb.tile([128, 160], f32, name="zt")
            nc.vector.memset(zt[:], 0.0)
            nc.sync.dma_start(
                out=s_loc.rearrange("(c p) f -> p c f", p=128),
                in_=zt[:].rearrange("p (c f) -> p c f", c=10))
            sxi = sb.tile([128, 2], mybir.dt.int32, name="sxi")
            nc.sync.dma_start(out=sxi[:], in_=sidx_ext[:, :])

            # ---- phase 3: layer-2 aggregation + exp + segment partials ----
            OC = 7
            sp = [pseg.tile([128, OUT], f32, name=f"segp{k}") for k in range(2)]
            gtiles.clear()
            load_group(0, "2")
            load_group(1, "2")
            gqs = {0: gather_tile(0, tab2v, "2")}
            for t in range(NT):
                wt = int(WT[t])
                wo4 = 4 * int(wt_off[t])
                if t % OC == 0:
                    ohf_c = ch.tile([128, OC * WSEG], f8, name=f"ohf{t}",
                                    tag="ohfc")
                    nc.sync.dma_start(
                        out=ohf_c[:].rearrange("p (u w) -> p u w", w=WSEG),
                        in_=ohf_ext[t * 128:(t + OC) * 128, :].rearrange(
                            "(u p) w -> p u w", p=128))
                gq = gqs.pop(t)
                if t + 1 < NT:
                    gqs[t + 1] = gather_tile(t + 1, tab2v, "2")
                gq4 = gq[:].rearrange("p w (q c) -> p w q c", c=ROW)
                msk = lp.tile([128, wt, 4], bf16, name=f"mk2_{t}", tag="mk")
                nc.vector.tensor_tensor(
                    out=msk[:],
                    in0=qvc[:, 0:4 * wt].rearrange("p (w q) -> p w q", q=4),
                    in1=sel_all[:, int(wt_off[t]):int(wt_off[t]) + wt][
                        :, :, None].to_broadcast([128, wt, 4]),
                    op=AL.is_equal)
                e1 = lp.tile([128, wt, 4], bf16, name=f"e1b_{t}", tag="e1")
                e2 = lp.tile([128, wt, 4], bf16, name=f"e2b_{t}", tag="e2")
                adc = ad2_all[:, t:t + 1]
                adcb = ad2b_all[:, t:t + 1]
                asv = gq4[:, :, :, 0:1]
                nc.scalar.activation(out=e1[:, :, :, None], in_=asv,
                                     func=EXP, scale=float(s2), bias=adc)
                nc.scalar.activation(out=e2[:, :, :, None], in_=asv,
                                     func=EXP, scale=float(0.2 * s2), bias=adcb)
                nc.vector.tensor_tensor(out=e1[:], in0=e1[:], in1=e2[:],
                                        op=AL.max)
                nc.vector.tensor_tensor(out=e1[:], in0=e1[:], in1=msk[:],
                                        op=AL.mult)
                den = sb.tile([128, 1], f32, name=f"den2_{t}", tag="den")
                nc.vector.reduce_sum(
                    out=den[:],
                    in_=e1[:].rearrange("p w q -> p (w q)")[:, None, :],
                    axis=XAX)
                macc = bp.tile([128, wt, 16], bf16, name=f"mA2_{t}", tag="mA")
                mtmp = bp.tile([128, wt, 16], bf16, name=f"mB2_{t}", tag="mB")
                for q in range(4):
                    nc.vector.tensor_tensor(
                        out=(macc if q == 0 else mtmp)[:],
                        in0=gq[:, :, q * ROW:q * ROW + 16],
                        in1=e1[:, :, q:q + 1].to_broadcast([128, wt, 16]),
                        op=AL.mult)
                    if q > 0:
                        nc.vector.tensor_tensor(out=macc[:], in0=macc[:],
                                                in1=mtmp[:], op=AL.add)
                num = sb.tile([128, OUT], f32, name=f"num2_{t}", tag="num")
                nc.vector.reduce_sum(out=num[:],
                                     in_=macc[:].rearrange("p w f -> p f w"),
                                     axis=XAX)
                rcp = sb.tile([128, 1], f32, name=f"rcp2_{t}", tag="rcp")
                nc.vector.reciprocal(out=rcp[:], in_=den[:])
                o2p = sb.tile([128, OUT], f32, name=f"o2p_{t}", tag="o2p")
                nc.vector.tensor_tensor(
                    out=o2p[:], in0=num[:],
                    in1=rcp[:, 0:1].to_broadcast([128, OUT]), op=AL.mult)
                # un-rotate: o2 = Q2^T num/den + b2
                trp = pp.tile([16, 128], f32, name=f"tr2{t}", tag="hp")
                nc.tensor.transpose(out=trp[:], in_=o2p[:], identity=ident[:])
                trb = sb.tile([16, 128], bf16, name=f"trb2{t}", tag="trb")
                nc.scalar.activation(out=trb[:], in_=trp[:], func=CPY)
                orr = pp.tile([128, OUT], f32, name=f"or{t}", tag="dp")
                nc.tensor.matmul(out=orr[:], lhsT=trb[:], rhs=qm2[:],
                                 start=True, stop=True)
                o2 = sb.tile([128, OUT], f32, name=f"o2_{t}", tag="o2")
                nc.vector.tensor_tensor(out=o2[:], in0=orr[:], in1=b2s[:],
                                        op=AL.add)
                nc.scalar.activation(out=e_all[:, OUT * t:OUT * (t + 1)],
                                     in_=o2[:], func=EXP)
                nc.scalar.activation(out=ebf_all[:, OUT * t:OUT * (t + 1)],
                                     in_=o2[:], func=EXP)
                for k in range(2):
                    o0 = (t % OC) * WSEG + k * 128
                    nc.tensor.matmul(
                        out=sp[k][:],
                        lhsT=ohf_c[:, o0:o0 + 128],
                        rhs=ebf_all[:, OUT * t:OUT * (t + 1)],
                        start=(t == 0), stop=(t == NT - 1))

            # ---- phase 4: combine segment sums across cores ---------------
            for k in range(2):
                spc = sb.tile([128, OUT], f32, name=f"spc{k}", tag="spc")
                nc.scalar.activation(out=spc[:], in_=sp[k][:], func=CPY)
                nc.gpsimd.indirect_dma_start(
                    out=s_loc[:, :],
                    out_offset=IOA(ap=sxi[:, k:k + 1], axis=0),
                    in_=spc[:], in_offset=None)

            nc.gpsimd.collective_compute(
                "AllReduce", AL.add, replica_groups=[list(range(NC))],
                ins=[s_loc.opt()], outs=[s_red.opt()])

            sw = []
            for k in range(2):
                swf = sb.tile([128, OUT], f32, name=f"swf{k}", tag="swf")
                nc.gpsimd.indirect_dma_start(
                    out=swf[:], out_offset=None,
                    in_=s_red[:, :],
                    in_offset=IOA(ap=sxi[:, k:k + 1], axis=0))
                swb = cpool.tile([128, OUT], bf16, name=f"sw{k}")
                nc.scalar.activation(out=swb[:], in_=swf[:], func=CPY)
                sw.append(swb)

            # ---- phase 5: divide, write out (batched by OC2 tiles) --------
            OC2 = 7
            for t0 in range(0, NT, OC2):
                oht_c = ch.tile([128, OC2 * WSEG], f8, name=f"oht{t0}",
                                tag="ohtc")
                nc.sync.dma_start(
                    out=oht_c[:].rearrange("p (u w) -> p u w", w=WSEG),
                    in_=oht_ext[t0 * 128:(t0 + OC2) * 128, :].rearrange(
                        "(u p) w -> p u w", p=128))
                dpb = pp.tile([128, OC2 * OUT], f32, name=f"dp{t0}", tag="dp")
                for u in range(OC2):
                    for k in range(2):
                        o0 = u * WSEG + k * 128
                        nc.tensor.matmul(out=dpb[:, u * OUT:(u + 1) * OUT],
                                         lhsT=oht_c[:, o0:o0 + 128],
                                         rhs=sw[k][:], start=(k == 0),
                                         stop=(k == 1))
                dd = sb.tile([128, OC2 * OUT], f32, name=f"dd{t0}", tag="dd")
                nc.vector.tensor_scalar_max(out=dd[:], in0=dpb[:], scalar1=1e-30)
                nc.vector.reciprocal(out=dd[:], in_=dd[:])
                nc.vector.tensor_tensor(
                    out=fo_all[:, OUT * t0:OUT * (t0 + OC2)],
                    in0=e_all[:, OUT * t0:OUT * (t0 + OC2)],
                    in1=dd[:], op=AL.mult)
            nc.sync.dma_start(
                out=out_ext[:, :].rearrange("(t p) f -> p t f", p=128),
                in_=fo_all[:].rearrange("p (t f) -> p t f", f=OUT))

    nc.compile()
    return nc


def kernel_impl(inputs, trace=False, tmpdir=None):
    from concourse.bass_utils import run_bass_kernel_spmd
    shared, per_core, asm = _preprocess(**inputs)
    nc = _build(shared)
    res = run_bass_kernel_spmd(nc, per_core, core_ids=list(range(NC)),
                               trace=trace, tmpdir=tmpdir)
    out = np.zeros((N, OUT), dtype=np.float32)
    for c in range(NC):
        o = np.asarray(res.results[c]["out"])
        m = asm["real"][c]
        out[asm["glb"][c][m]] = o[m]
    return out, res


def kernel(**inputs):
    out, _ = kernel_impl(inputs, trace=False)
    return out


# revision 37
# speedup vs baseline: 1.0130x; 1.0130x over previous
"""TRN2 Bass kernel for 2-layer GAT + grouped softmax (nn_Actor_1881195675935).

8-core SPMD, nodes sharded contiguously (12500/core, padded to 12544 = 98
tiles of 128); edges live with the owner of their dst node in an ELLPACK
layout (partition = dst node, free axis = edge slots).

v4 design (vs v3): the v3 trace showed the gather descriptor stream draining
at ~2.3 ns/desc — the phases are bound by descriptor count, not DVE.  v4 cuts
descriptors 19% and removes the two-window machinery entirely:
  * per-head feature-basis rotation: rows store h' = Q_h h with Q_h chosen so
    a_src = s_h * h'[0].  Node rows shrink to 32 bf16 (64B), so a 256B gather
    row holds FOUR nodes and quad indices (pos>>2 < 25088) fit int16 in a
    SINGLE window — ELLPACK needs no A/B split (totWT 4032 -> 3270, one
    gather call per tile).  Aggregated num' is un-rotated per tile with a
    tiny PE matmul (blockdiag(Q_h)) before bias+relu.
  * logits come straight off the gathered bytes on the ACT engine:
    e = exp(a's * s_h + a_d) via activation(EXP, scale=s_h, bias=ad_ap) — no
    DVE z materialisation; 4-way sub-row masks (multiplicative, bf16) pick
    the real half.
  * both layers share one resident gidx/mask set; AllGathers halve (6.4MB).
All v3 scheduling fixes kept: no 2-port DVE ops in gather phases, ACT-engine
casts (one act-table set), batched HWDGE transfers.
"""

import sys

sys.path.insert(0, "/opt/trn_rl_repo")

import numpy as np
import ml_dtypes  # noqa: F401

N = 100000
NPC_REAL = 12500
NPC = 12544               # = 98 * 128
NT = 98
NC = 8
F_IN = 128
H1, C1 = 2, 16
OUT = 16
WSEG = 256
ROW = 32                  # bf16 elements per node row (64B); quad row = 256B
NQROWS = NC * NPC // 4    # 25088 quad rows (single int16 window)
CHUNK = 6                 # max slot columns per dma_gather call (768 descs;
                          # two calls fit one SWDGE queue ring, so Q7 never
                          # blocks inside a call and all 4 queues drain in
                          # parallel)
PADQ = (2 * NPC + NPC_REAL) // 4   # all-zero pad quad (core 2 spares)


def _orth_rot(a):
    """Orthogonal Q (16x16) with Q[0,:] ~ a/||a||; returns (Q, s) such that
    a @ h == s * (Q @ h)[0] for all h."""
    a = np.asarray(a, np.float64)
    M = np.eye(len(a))
    M[:, 0] = a
    Qc, _ = np.linalg.qr(M)
    s = float(Qc[:, 0] @ a)
    return Qc.T.astype(np.float64), s


def _preprocess(x, edge_index, index, W1, att_src1, att_dst1, b1,
                W2, att_src2, att_dst2, b2):
    f32 = np.float32
    src = np.asarray(edge_index[0], dtype=np.int64)
    dst = np.asarray(edge_index[1], dtype=np.int64)
    loops = np.arange(N, dtype=np.int64)
    src = np.concatenate([src, loops]).astype(np.int64)
    dst = np.concatenate([dst, loops]).astype(np.int64)

    owner_dst = dst // NPC_REAL
    ldst = dst - owner_dst * NPC_REAL

    # per-core relabel: degree-sorted descending so each tile's 128 lanes have
    # near-equal degree (minimises ELLPACK padding via max-over-lanes)
    counts_deg = np.bincount(owner_dst * NPC + ldst, minlength=NC * NPC)
    counts_deg = counts_deg.reshape(NC, NPC)
    orders = np.zeros((NC, NPC), dtype=np.int64)
    for c in range(NC):
        orders[c] = np.argsort(-counts_deg[c], kind="stable")
    inv_orders = np.argsort(orders, axis=1)
    pos = np.zeros(N, dtype=np.int64)
    ar = np.arange(NPC_REAL)
    for c in range(NC):
        pos[c * NPC_REAL + ar] = c * NPC + inv_orders[c][ar]

    spos = pos[src]
    nid = owner_dst * NPC + inv_orders[owner_dst, ldst]

    deg = np.bincount(nid, minlength=NC * NPC)
    WT = deg.reshape(NC, NT, 128).max(axis=(0, 2)).astype(np.int64)  # [NT]
    totWT = int(WT.sum())
    wt_off = np.concatenate([[0], np.cumsum(WT)]).astype(np.int64)

    calls = []  # (tile, col0, ncols)
    for t in range(NT):
        c0 = 0
        while c0 < int(WT[t]):
            cn = min(CHUNK, int(WT[t]) - c0)
            calls.append((t, c0, cn))
            c0 += cn
    idxw_off = [0]
    for (_t, _c0, cn) in calls:
        idxw_off.append(idxw_off[-1] + 8 * cn)
    IDXW = idxw_off[-1]

    # sorted edge runs per node
    eorder = np.lexsort((spos, nid))
    s_spos = spos[eorder]
    run_starts = np.zeros(NC * NPC + 1, dtype=np.int64)
    np.cumsum(deg, out=run_starts[1:])

    bf16 = ml_dtypes.bfloat16
    gidx = np.zeros((NC, 128, IDXW), dtype=np.int16)
    # per-slot sub-row selector (-1 = unused slot); device materialises the
    # 4-way one-hot mask via is_equal against the qv constant
    sel = np.full((NC, 128, totWT), -1.0, dtype=np.float32)

    for c in range(NC):
        nodes_all = c * NPC + np.arange(NPC)
        r0 = run_starts[nodes_all]
        d_all = deg[nodes_all]
        for t in range(NT):
            wt = int(WT[t])
            flat = np.full((wt, 128), PADQ, dtype=np.int64)
            sub = np.zeros((wt, 128), dtype=np.int64)
            used = np.zeros((wt, 128), dtype=bool)
            for lane in range(128):
                n = t * 128 + lane
                d = int(d_all[n])
                if d == 0:
                    continue
                e0 = r0[n]
                sp = s_spos[e0:e0 + d]
                flat[:d, lane] = sp >> 2
                sub[:d, lane] = sp & 3
                used[:d, lane] = True
            sv = np.full((wt, 128), -1.0, dtype=np.float32)
            sv[used] = sub[used]
            dead = ~used.any(axis=0)
            sv[0, dead] = 0.0  # pad quad, sub 0: zero node -> den>0, num+=0
            sel[c, :, wt_off[t]:wt_off[t] + wt] = sv.T
            assert flat.max() < NQROWS and flat.min() >= 0
            for ci, (tt, col0, cn) in enumerate(calls):
                if tt != t:
                    continue
                blk = flat[col0:col0 + cn]
                w16 = blk.reshape(-1, 16).T.astype(np.int16)
                gidx[c, :, idxw_off[ci]:idxw_off[ci + 1]] = np.tile(w16, (8, 1))

    W1 = np.asarray(W1, np.float64); W2 = np.asarray(W2, np.float64)
    as1 = np.asarray(att_src1, np.float64); ad1 = np.asarray(att_dst1, np.float64)
    as2 = np.asarray(att_src2, np.float64); ad2 = np.asarray(att_dst2, np.float64)
    Q1 = []; s1 = []
    for h in range(H1):
        Qh, sh = _orth_rot(as1[h])
        Q1.append(Qh); s1.append(sh)
    W1q = np.concatenate(
        [W1[:, h * C1:(h + 1) * C1] @ Q1[h].T for h in range(H1)], axis=1)
    vd1 = np.stack([W1[:, h * C1:(h + 1) * C1] @ ad1[h] for h in range(H1)], 1)
    wcat1 = np.concatenate([W1q, vd1], axis=1).astype(bf16)      # [128, 34]
    qb1 = np.zeros((2 * C1, 2 * C1))
    for h in range(H1):
        qb1[h * C1:(h + 1) * C1, h * C1:(h + 1) * C1] = Q1[h]
    qb1 = qb1.astype(bf16)                                       # [32, 32]

    Q2, s2 = _orth_rot(as2[0])
    W2q = W2 @ Q2.T
    vd2 = (W2 @ ad2[0])[:, None]
    wcat2 = np.concatenate([W2q, vd2], axis=1).astype(bf16)      # [32, 17]
    qm2 = Q2.astype(bf16)                                        # [16, 16]

    x = np.asarray(x, f32)
    xT = np.zeros((NC, F_IN, NPC), dtype=bf16)
    glb = np.zeros((NC, NPC), dtype=np.int64)
    real = np.zeros((NC, NPC), dtype=bool)
    for c in range(NC):
        ol = orders[c]
        is_real = ol < NPC_REAL
        g = np.where(is_real, c * NPC_REAL + np.minimum(ol, NPC_REAL - 1), 0)
        xT[c] = np.where(is_real[:, None], x[g], 0.0).T.astype(bf16)
        glb[c] = g
        real[c] = is_real

    index = np.asarray(index, np.int64)
    seg = np.zeros((NC, NPC), dtype=np.int64)
    g0 = np.zeros(NC, dtype=np.int64)
    for c in range(NC):
        seg[c] = np.where(real[c], index[glb[c]], 0)
        s = seg[c][real[c]]
        g0[c] = s.min()
        assert s.max() - s.min() < WSEG, "segment window exceeds WSEG"
    f8 = ml_dtypes.float8_e4m3
    ohf = np.zeros((NC, NT * 128, WSEG), dtype=f8)
    oht = np.zeros((NC, NT * 128, WSEG), dtype=f8)
    for c in range(NC):
        for t in range(NT):
            sl = seg[c, t * 128:(t + 1) * 128] - g0[c]
            m = real[c, t * 128:(t + 1) * 128]
            oh = np.zeros((128, WSEG), dtype=np.float32)
            oh[np.arange(128)[m], sl[m]] = 1.0
            ohf[c, t * 128:(t + 1) * 128] = oh.astype(f8)
            ohtk = np.concatenate([oh[:, :128].T, oh[:, 128:].T], axis=1)
            oht[c, t * 128:(t + 1) * 128] = ohtk.astype(f8)

    sidx = np.zeros((NC, 128, 2), dtype=np.int32)
    for c in range(NC):
        for k in range(2):
            sidx[c, :, k] = g0[c] + k * 128 + np.arange(128)

    WTMAX = int(WT.max())
    qv = np.tile(np.arange(4, dtype=np.float32), WTMAX)
    qv = np.tile(qv[None, :], (128, 1)).astype(bf16)
    b1t = np.tile(np.asarray(b1, f32)[None, :], (128, 1)).astype(f32)
    b2t = np.tile(np.asarray(b2, f32)[None, :], (128, 1)).astype(f32)

    per_core = [{
        "xT": np.ascontiguousarray(xT[c]),
        "wcat1": wcat1, "wcat2": wcat2, "qb1": qb1, "qm2": qm2,
        "b1t": b1t, "b2t": b2t,
        "gidx": np.ascontiguousarray(gidx[c]),
        "sel": np.ascontiguousarray(sel[c].astype(bf16)),
        "qv": qv,
        "ohf": np.ascontiguousarray(ohf[c]),
        "oht": np.ascontiguousarray(oht[c]),
        "sidx": np.ascontiguousarray(sidx[c]),
    } for c in range(NC)]
    shared = {"WT": WT, "calls": calls, "idxw_off": idxw_off,
              "IDXW": IDXW, "wt_off": wt_off, "totWT": totWT,
              "s1": s1, "s2": s2, "WTMAX": int(WT.max())}
    asm = {"glb": glb, "real": real}
    return shared, per_core, asm


def _build(shared):
    import concourse.bass as bass
    import concourse.bacc as bacc
    import concourse.tile as tile
    from concourse import mybir, library_config
    from concourse.masks import make_identity

    calls = shared["calls"]
    idxw_off = shared["idxw_off"]; IDXW = shared["IDXW"]
    wt_off = shared["wt_off"]; totWT = shared["totWT"]; WT = shared["WT"]
    s1 = shared["s1"]; s2 = shared["s2"]
    f32 = mybir.dt.float32
    bf16 = mybir.dt.bfloat16
    f8 = mybir.dt.float8e4
    i16 = mybir.dt.int16
    AL = mybir.AluOpType
    EXP = mybir.ActivationFunctionType.Exp
    CPY = mybir.ActivationFunctionType.Copy
    RELU = mybir.ActivationFunctionType.Relu
    IOA = bass.IndirectOffsetOnAxis
    XAX = mybir.AxisListType.X

    nc = bacc.Bacc("TRN2", target_bir_lowering=False, debug=False,
                   num_devices=NC, num_swdge_queues=4)

    xT_ext = nc.dram_tensor("xT", [F_IN, NPC], bf16, kind="ExternalInput")
    wcat1_ext = nc.dram_tensor("wcat1", [F_IN, 34], bf16, kind="ExternalInput")
    wcat2_ext = nc.dram_tensor("wcat2", [32, 17], bf16, kind="ExternalInput")
    qb1_ext = nc.dram_tensor("qb1", [32, 32], bf16, kind="ExternalInput")
    qm2_ext = nc.dram_tensor("qm2", [16, 16], bf16, kind="ExternalInput")
    b1_ext = nc.dram_tensor("b1t", [128, 32], f32, kind="ExternalInput")
    b2_ext = nc.dram_tensor("b2t", [128, 16], f32, kind="ExternalInput")
    gidx_ext = nc.dram_tensor("gidx", [128, IDXW], i16, kind="ExternalInput")
    sel_ext = nc.dram_tensor("sel", [128, totWT], bf16, kind="ExternalInput")
    qv_ext = nc.dram_tensor("qv", [128, 4 * shared["WTMAX"]], bf16,
                            kind="ExternalInput")
    ohf_ext = nc.dram_tensor("ohf", [NT * 128, WSEG], f8, kind="ExternalInput")
    oht_ext = nc.dram_tensor("oht", [NT * 128, WSEG], f8, kind="ExternalInput")
    sidx_ext = nc.dram_tensor("sidx", [128, 2], mybir.dt.int32, kind="ExternalInput")
    out_ext = nc.dram_tensor("out", [NPC, OUT], f32, kind="ExternalOutput")

    with tile.TileContext(nc) as tc:
        with (
            tc.tile_pool(name="dram", bufs=1, space="DRAM") as dr,
            tc.tile_pool(name="const", bufs=1) as cpool,
            tc.tile_pool(name="res", bufs=1) as rp,
            tc.tile_pool(name="gat", bufs=5) as gp,
            tc.tile_pool(name="logit", bufs=3) as lp,
            tc.tile_pool(name="big", bufs=2) as bp,
            tc.tile_pool(name="tiny", bufs=4) as sb,
            tc.tile_pool(name="chunk", bufs=2) as ch,
            tc.tile_pool(name="psum", bufs=2, space="PSUM") as pp,
            tc.tile_pool(name="psum_seg", bufs=1, space="PSUM") as pseg,
        ):
            tab1_loc = dr.tile([NPC, ROW], bf16, name="tab1_loc")
            tab2_loc = dr.tile([NPC, ROW], bf16, name="tab2_loc")
            tab1_full = dr.tile([NC * NPC, ROW], bf16, name="tab1_full",
                                addr_space="Shared")
            tab2_full = dr.tile([NC * NPC, ROW], bf16, name="tab2_full",
                                addr_space="Shared")
            s_loc = dr.tile([1280, OUT], f32, name="s_loc")
            s_red = dr.tile([1280, OUT], f32, name="s_red", addr_space="Shared")

            tab1v = tab1_full[:].rearrange("(r four) c -> r (four c)", four=4)
            tab2v = tab2_full[:].rearrange("(r four) c -> r (four c)", four=4)

            nc.gpsimd.load_library(library_config.mlp)

            ident = cpool.tile([128, 128], f32, name="ident")
            make_identity(nc, ident[:])
            identb = cpool.tile([128, 128], bf16, name="identb")
            nc.scalar.activation(out=identb[:], in_=ident[:], func=CPY)
            wc1 = cpool.tile([F_IN, 34], bf16, name="wc1")
            nc.sync.dma_start(out=wc1[:], in_=wcat1_ext[:, :])
            wc2 = cpool.tile([32, 17], bf16, name="wc2")
            nc.sync.dma_start(out=wc2[:], in_=wcat2_ext[:, :])
            qb1 = cpool.tile([32, 32], bf16, name="qb1")
            nc.sync.dma_start(out=qb1[:], in_=qb1_ext[:, :])
            qm2 = cpool.tile([16, 16], bf16, name="qm2")
            nc.sync.dma_start(out=qm2[:], in_=qm2_ext[:, :])
            b1s = cpool.tile([128, 32], f32, name="b1s")
            nc.sync.dma_start(out=b1s[:], in_=b1_ext[:, :])
            b2s = cpool.tile([128, 16], f32, name="b2s")
            nc.sync.dma_start(out=b2s[:], in_=b2_ext[:, :])

            # residents
            gidx_all = rp.tile([128, IDXW], i16, name="gidx_all")
            nc.sync.dma_start(out=gidx_all[:], in_=gidx_ext[:, :])
            sel_all = rp.tile([128, totWT], bf16, name="sel_all")
            nc.sync.dma_start(out=sel_all[:], in_=sel_ext[:, :])
            qvc = cpool.tile([128, 4 * shared["WTMAX"]], bf16, name="qvc")
            nc.sync.dma_start(out=qvc[:], in_=qv_ext[:, :])
            stage_all = rp.tile([128, NT * ROW], bf16, name="stage_all")
            nc.vector.memset(stage_all[:], 0.0)
            ad1_all = rp.tile([128, NT * 2], f32, name="ad1_all")
            ad1b_all = rp.tile([128, NT * 2], f32, name="ad1b_all")
            ad2_all = rp.tile([128, NT], f32, name="ad2_all")
            ad2b_all = rp.tile([128, NT], f32, name="ad2b_all")
            x2_all = rp.tile([128, NT * 32], bf16, name="x2_all")
            e_all = rp.tile([128, NT * OUT], f32, name="e_all")
            ebf_all = rp.tile([128, NT * OUT], bf16, name="ebf_all")
            fo_all = rp.tile([128, NT * OUT], f32, name="fo_all")

            calls_by_tile = {}
            for ci, (t, col0, cn) in enumerate(calls):
                calls_by_tile.setdefault(t, []).append((ci, col0, cn))
            qctr = [0]

            def gather_tile(t, tabv, lname):
                wt = int(WT[t])
                gq = gp.tile([128, wt, 4 * ROW], bf16, name=f"g{lname}_{t}",
                             tag="gq")
                for (ci, col0, cn) in calls_by_tile[t]:
                    nidx = 128 * cn
                    nc.gpsimd.dma_gather(
                        gq[:, col0:col0 + cn, :],
                        tabv[0:NQROWS, :],
                        gidx_all[:, idxw_off[ci]:idxw_off[ci + 1]],
                        nidx, nidx, 4 * ROW, queue_num=qctr[0] % 4,
                        single_packet=False)
                    qctr[0] += 1
                return gq

            # ---- phase 0: layer-1 node rows -------------------------------
            XC = 7  # tiles per x chunk
            for t in range(NT):
                if t % XC == 0:
                    xt_c = ch.tile([128, XC * 128], bf16, name=f"xt{t}",
                                   tag="xtc")
                    nc.sync.dma_start(
                        out=xt_c[:],
                        in_=xT_ext[:, t * 128:(t + XC) * 128])
                hp = pp.tile([128, 34], f32, name=f"hp{t}", tag="hp")
                nc.tensor.matmul(out=hp[:],
                                 lhsT=xt_c[:, (t % XC) * 128:(t % XC + 1) * 128],
                                 rhs=wc1[:], start=True, stop=True)
                nc.scalar.activation(out=ad1_all[:, 2 * t:2 * t + 2],
                                     in_=hp[:, 32:34], func=CPY)
                nc.scalar.activation(out=ad1b_all[:, 2 * t:2 * t + 2],
                                     in_=hp[:, 32:34], func=CPY, scale=0.2)
                nc.scalar.activation(out=stage_all[:, ROW * t:ROW * t + 32],
                                     in_=hp[:, 0:32], func=CPY)

            nc.sync.dma_start(
                out=tab1_loc[:].rearrange("(t p) c -> p t c", p=128),
                in_=stage_all[:].rearrange("p (t c) -> p t c", c=ROW))
            nc.gpsimd.collective_compute(
                "AllGather", AL.bypass, replica_groups=[list(range(NC))],
                ins=[tab1_loc.opt()], outs=[tab1_full.opt()])

            # ---- phase 1: layer-1 aggregation (+ fused layer-2 rows) ------
            gqs = {0: gather_tile(0, tab1v, "1")}
            for t in range(NT):
                wt = int(WT[t])
                wo4 = 4 * int(wt_off[t])
                gq = gqs.pop(t)
                if t + 1 < NT:
                    gqs[t + 1] = gather_tile(t + 1, tab1v, "1")
                gq4 = gq[:].rearrange("p w (q c) -> p w q c", c=ROW)
                msk = lp.tile([128, wt, 4], bf16, name=f"mk1_{t}", tag="mk")
                nc.vector.tensor_tensor(
                    out=msk[:],
                    in0=qvc[:, 0:4 * wt].rearrange("p (w q) -> p w q", q=4),
                    in1=sel_all[:, int(wt_off[t]):int(wt_off[t]) + wt][
                        :, :, None].to_broadcast([128, wt, 4]),
                    op=AL.is_equal)
                e1 = lp.tile([128, 2 * wt, 4], bf16, name=f"e1a_{t}", tag="e1")
                e2 = lp.tile([128, 2 * wt, 4], bf16, name=f"e2a_{t}", tag="e2")
                for h in range(2):
                    adc = ad1_all[:, 2 * t + h:2 * t + h + 1]
                    adcb = ad1b_all[:, 2 * t + h:2 * t + h + 1]
                    asv = gq4[:, :, :, 16 * h:16 * h + 1]
                    nc.scalar.activation(
                        out=e1[:, h * wt:(h + 1) * wt, :, None], in_=asv,
                        func=EXP, scale=float(s1[h]), bias=adc)
                    nc.scalar.activation(
                        out=e2[:, h * wt:(h + 1) * wt, :, None], in_=asv,
                        func=EXP, scale=float(0.2 * s1[h]), bias=adcb)
                # exp(leaky_relu(z)) == max(exp(z), exp(0.2*z)); then mask
                nc.vector.tensor_tensor(out=e1[:], in0=e1[:], in1=e2[:],
                                        op=AL.max)
                e1h = e1[:].rearrange("p (h w) q -> p h w q", h=2)
                nc.vector.tensor_tensor(
                    out=e1h, in0=e1h,
                    in1=msk[:, None, :, :].to_broadcast([128, 2, wt, 4]),
                    op=AL.mult)
                den = sb.tile([128, 2], f32, name=f"den1_{t}", tag="den")
                nc.vector.reduce_sum(
                    out=den[:],
                    in_=e1[:].rearrange("p (h w) q -> p h (w q)", h=2),
                    axis=XAX)
                macc = bp.tile([128, wt, 32], bf16, name=f"mA1_{t}", tag="mA")
                mtmp = bp.tile([128, wt, 32], bf16, name=f"mB1_{t}", tag="mB")
                e1p = e1[:].rearrange("p (h w) q -> p w h q", h=2)
                for q in range(4):
                    dst4 = (macc if q == 0 else mtmp)[:].rearrange(
                        "p w (h f) -> p w h f", h=2)
                    nc.vector.tensor_tensor(
                        out=dst4,
                        in0=gq[:, :, q * ROW:(q + 1) * ROW].rearrange(
                            "p w (h f) -> p w h f", h=2),
                        in1=e1p[:, :, :, q:q + 1].to_broadcast([128, wt, 2, 16]),
                        op=AL.mult)
                    if q > 0:
                        nc.vector.tensor_tensor(out=macc[:], in0=macc[:],
                                                in1=mtmp[:], op=AL.add)
                num = sb.tile([128, 32], f32, name=f"num1_{t}", tag="num")
                nc.vector.reduce_sum(out=num[:],
                                     in_=macc[:].rearrange("p w f -> p f w"),
                                     axis=XAX)
                rcp = sb.tile([128, 2], f32, name=f"rcp1_{t}", tag="rcp")
                nc.vector.reciprocal(out=rcp[:], in_=den[:])
                x2t = sb.tile([128, 32], f32, name=f"x2_{t}", tag="x2t")
                for h in range(2):
                    nc.vector.tensor_tensor(
                        out=x2t[:, 16 * h:16 * h + 16],
                        in0=num[:, 16 * h:16 * h + 16],
                        in1=rcp[:, h:h + 1].to_broadcast([128, 16]),
                        op=AL.mult)
                # un-rotate: x2 = relu(Q^T num/den + b1)
                trp = pp.tile([32, 128], f32, name=f"trp{t}", tag="hp")
                nc.tensor.transpose(out=trp[:], in_=x2t[:], identity=ident[:])
                trb = sb.tile([32, 128], bf16, name=f"trb{t}", tag="trb")
                nc.scalar.activation(out=trb[:], in_=trp[:], func=CPY)
                xr = pp.tile([128, 32], f32, name=f"xr{t}", tag="dp")
                nc.tensor.matmul(out=xr[:], lhsT=trb[:], rhs=qb1[:],
                                 start=True, stop=True)
                x2i = sb.tile([128, 32], f32, name=f"x2i_{t}", tag="x2i")
                nc.vector.tensor_tensor(out=x2i[:], in0=xr[:], in1=b1s[:],
                                        op=AL.add)
                nc.scalar.activation(out=x2_all[:, 32 * t:32 * (t + 1)],
                                     in_=x2i[:], func=RELU)
                # fused layer-2 node-row computation for this tile
                x2tp = pp.tile([32, 128], bf16, name=f"x2tp{t}", tag="hp")
                nc.tensor.transpose(out=x2tp[:],
                                    in_=x2_all[:, 32 * t:32 * (t + 1)],
                                    identity=identb[:])
                x2ts = sb.tile([32, 128], bf16, name=f"x2ts{t}", tag="trb")
                nc.scalar.activation(out=x2ts[:], in_=x2tp[:], func=CPY)
                h2p = pp.tile([128, 17], f32, name=f"h2p{t}", tag="dp")
                nc.tensor.matmul(out=h2p[:], lhsT=x2ts[:], rhs=wc2[:],
                                 start=True, stop=True)
                nc.scalar.activation(out=ad2_all[:, t:t + 1],
                                     in_=h2p[:, 16:17], func=CPY)
                nc.scalar.activation(out=ad2b_all[:, t:t + 1],
                                     in_=h2p[:, 16:17], func=CPY, scale=0.2)
                nc.scalar.activation(out=stage_all[:, ROW * t:ROW * t + 16],
                                     in_=h2p[:, 0:16], func=CPY)


            nc.sync.dma_start(
                out=tab2_loc[:].rearrange("(t p) c -> p t c", p=128),
                in_=stage_all[:].rearrange("p (t c) -> p t c", c=ROW))
            nc.gpsimd.collective_compute(
                "AllGather", AL.bypass, replica_groups=[list(range(NC))],
                ins=[tab2_loc.opt()], outs=[tab2_full.opt()])

            # ---- phase 4 setup (hoisted; overlaps phase 3) -----------------
            zt = sb.tile([128, 160], f32, name="zt")
            nc.vector.memset(zt[:], 0.0)
            nc.sync.dma_start(
                out=s_loc.rearrange("(c p) f -> p c f", p=128),
                in_=zt[:].rearrange("p (c f) -> p c f", c=10))
            sxi = sb.tile([128, 2], mybir.dt.int32, name="sxi")
            nc.sync.dma_start(out=sxi[:], in_=sidx_ext[:, :])

            # ---- phase 3: layer-2 aggregation + exp + segment partials ----
            OC = 7
            sp = [pseg.tile([128, OUT], f32, name=f"segp{k}") for k in range(2)]
            gqs = {0: gather_tile(0, tab2v, "2")}
            for t in range(NT):
                wt = int(WT[t])
                wo4 = 4 * int(wt_off[t])
                if t % OC == 0:
                    ohf_c = ch.tile([128, OC * WSEG], f8, name=f"ohf{t}",
                                    tag="ohfc")
                    nc.sync.dma_start(
                        out=ohf_c[:].rearrange("p (u w) -> p u w", w=WSEG),
                        in_=ohf_ext[t * 128:(t + OC) * 128, :].rearrange(
                            "(u p) w -> p u w", p=128))
                gq = gqs.pop(t)
                if t + 1 < NT:
                    gqs[t + 1] = gather_tile(t + 1, tab2v, "2")
                gq4 = gq[:].rearrange("p w (q c) -> p w q c", c=ROW)
                msk = lp.tile([128, wt, 4], bf16, name=f"mk2_{t}", tag="mk")
                nc.vector.tensor_tensor(
                    out=msk[:],
                    in0=qvc[:, 0:4 * wt].rearrange("p (w q) -> p w q", q=4),
                    in1=sel_all[:, int(wt_off[t]):int(wt_off[t]) + wt][
                        :, :, None].to_broadcast([128, wt, 4]),
                    op=AL.is_equal)
                e1 = lp.tile([128, wt, 4], bf16, name=f"e1b_{t}", tag="e1")
                e2 = lp.tile([128, wt, 4], bf16, name=f"e2b_{t}", tag="e2")
                adc = ad2_all[:, t:t + 1]
                adcb = ad2b_all[:, t:t + 1]
                asv = gq4[:, :, :, 0:1]
                nc.scalar.activation(out=e1[:, :, :, None], in_=asv,
                                     func=EXP, scale=float(s2), bias=adc)
                nc.scalar.activation(out=e2[:, :, :, None], in_=asv,
                                     func=EXP, scale=float(0.2 * s2), bias=adcb)
                nc.vector.tensor_tensor(out=e1[:], in0=e1[:], in1=e2[:],
                                        op=AL.max)
                nc.vector.tensor_tensor(out=e1[:], in0=e1[:], in1=msk[:],
                                        op=AL.mult)
                den = sb.tile([128, 1], f32, name=f"den2_{t}", tag="den")
                nc.vector.reduce_sum(
                    out=den[:],
                    in_=e1[:].rearrange("p w q -> p (w q)")[:, None, :],
                    axis=XAX)
                macc = bp.tile([128, wt, 16], bf16, name=f"mA2_{t}", tag="mA")
                mtmp = bp.tile([128, wt, 16], bf16, name=f"mB2_{t}", tag="mB")
                for q in range(4):
                    nc.vector.tensor_tensor(
                        out=(macc if q == 0 else mtmp)[:],
                        in0=gq[:, :, q * ROW:q * ROW + 16],
                        in1=e1[:, :, q:q + 1].to_broadcast([128, wt, 16]),
                        op=AL.mult)
                    if q > 0:
                        nc.vector.tensor_tensor(out=macc[:], in0=macc[:],
                                                in1=mtmp[:], op=AL.add)
                num = sb.tile([128, OUT], f32, name=f"num2_{t}", tag="num")
                nc.vector.reduce_sum(out=num[:],
                                     in_=macc[:].rearrange("p w f -> p f w"),
                                     axis=XAX)
                rcp = sb.tile([128, 1], f32, name=f"rcp2_{t}", tag="rcp")
                nc.vector.reciprocal(out=rcp[:], in_=den[:])
                o2p = sb.tile([128, OUT], f32, name=f"o2p_{t}", tag="o2p")
                nc.vector.tensor_tensor(
                    out=o2p[:], in0=num[:],
                    in1=rcp[:, 0:1].to_broadcast([128, OUT]), op=AL.mult)
                # un-rotate: o2 = Q2^T num/den + b2
                trp = pp.tile([16, 128], f32, name=f"tr2{t}", tag="hp")
                nc.tensor.transpose(out=trp[:], in_=o2p[:], identity=ident[:])
                trb = sb.tile([16, 128], bf16, name=f"trb2{t}", tag="trb")
                nc.scalar.activation(out=trb[:], in_=trp[:], func=CPY)
                orr = pp.tile([128, OUT], f32, name=f"or{t}", tag="dp")
                nc.tensor.matmul(out=orr[:], lhsT=trb[:], rhs=qm2[:],
                                 start=True, stop=True)
                o2 = sb.tile([128, OUT], f32, name=f"o2_{t}", tag="o2")
                nc.vector.tensor_tensor(out=o2[:], in0=orr[:], in1=b2s[:],
                                        op=AL.add)
                nc.scalar.activation(out=e_all[:, OUT * t:OUT * (t + 1)],
                                     in_=o2[:], func=EXP)
                nc.scalar.activation(out=ebf_all[:, OUT * t:OUT * (t + 1)],
                                     in_=o2[:], func=EXP)
                for k in range(2):
                    o0 = (t % OC) * WSEG + k * 128
                    nc.tensor.matmul(
                        out=sp[k][:],
                        lhsT=ohf_c[:, o0:o0 + 128],
                        rhs=ebf_all[:, OUT * t:OUT * (t + 1)],
                        start=(t == 0), stop=(t == NT - 1))

            # ---- phase 4: combine segment sums across cores ---------------
            for k in range(2):
                spc = sb.tile([128, OUT], f32, name=f"spc{k}", tag="spc")
                nc.scalar.activation(out=spc[:], in_=sp[k][:], func=CPY)
                nc.gpsimd.indirect_dma_start(
                    out=s_loc[:, :],
                    out_offset=IOA(ap=sxi[:, k:k + 1], axis=0),
                    in_=spc[:], in_offset=None)

            nc.gpsimd.collective_compute(
                "AllReduce", AL.add, replica_groups=[list(range(NC))],
                ins=[s_loc.opt()], outs=[s_red.opt()])

            sw = []
            for k in range(2):
                swf = sb.tile([128, OUT], f32, name=f"swf{k}", tag="swf")
                nc.gpsimd.indirect_dma_start(
                    out=swf[:], out_offset=None,
                    in_=s_red[:, :],
                    in_offset=IOA(ap=sxi[:, k:k + 1], axis=0))
                swb = cpool.tile([128, OUT], bf16, name=f"sw{k}")
                nc.scalar.activation(out=swb[:], in_=swf[:], func=CPY)
                sw.append(swb)

            # ---- phase 5: divide, write out (batched by OC2 tiles) --------
            OC2 = 7
            for t0 in range(0, NT, OC2):
                oht_c = ch.tile([128, OC2 * WSEG], f8, name=f"oht{t0}",
                                tag="ohtc")
                nc.sync.dma_start(
                    out=oht_c[:].rearrange("p (u w) -> p u w", w=WSEG),
                    in_=oht_ext[t0 * 128:(t0 + OC2) * 128, :].rearrange(
                        "(u p) w -> p u w", p=128))
                dpb = pp.tile([128, OC2 * OUT], f32, name=f"dp{t0}", tag="dp")
                for u in range(OC2):
                    for k in range(2):
                        o0 = u * WSEG + k * 128
                        nc.tensor.matmul(out=dpb[:, u * OUT:(u + 1) * OUT],
                                         lhsT=oht_c[:, o0:o0 + 128],
                                         rhs=sw[k][:], start=(k == 0),
                                         stop=(k == 1))
                dd = sb.tile([128, OC2 * OUT], f32, name=f"dd{t0}", tag="dd")
                nc.vector.tensor_scalar_max(out=dd[:], in0=dpb[:], scalar1=1e-30)
                nc.vector.reciprocal(out=dd[:], in_=dd[:])
                nc.vector.tensor_tensor(
                    out=fo_all[:, OUT * t0:OUT * (t0 + OC2)],
                    in0=e_all[:, OUT * t0:OUT * (t0 + OC2)],
                    in1=dd[:], op=AL.mult)
            nc.sync.dma_start(
                out=out_ext[:, :].rearrange("(t p) f -> p t f", p=128),
                in_=fo_all[:].rearrange("p (t f) -> p t f", f=OUT))

    nc.compile()
    return nc


def kernel_impl(inputs, trace=False, tmpdir=None):
    from concourse.bass_utils import run_bass_kernel_spmd
    shared, per_core, asm = _preprocess(**inputs)
    nc = _build(shared)
    res = run_bass_kernel_spmd(nc, per_core, core_ids=list(range(NC)),
                               trace=trace, tmpdir=tmpdir)
    out = np.zeros((N, OUT), dtype=np.float32)
    for c in range(NC):
        o = np.asarray(res.results[c]["out"])
        m = asm["real"][c]
        out[asm["glb"][c][m]] = o[m]
    return out, res


def kernel(**inputs):
    out, _ = kernel_impl(inputs, trace=False)
    return out
